# revision 1
# baseline (speedup 1.0000x reference)
"""Multi-head causal attention (B=256,T=256,E=384,H=6,D=64) on 8 trn2 cores.

Sharding: data-parallel over batch (32 items per core), no collectives.

Per-core pipeline (items processed in pairs for the QKV phase):
  x [256,384] --PE transpose--> xT [384, 512] fp16 (pair)
  QT/KT = w.T @ xT   [384, 512] fp16  (heads stacked on partitions)
  V  = xT.T @ wv     [256, 390] fp16  per item, layout [k, h*65+d] with a
       ones column at h*65+64 (fuses row-sum into the AV matmul)
  per head h (per item):
    ST[k,q] = KT_h.T @ QT_h directly (causal: 2 blocks, skip upper rect),
              additive -60000 mask on diagonal blocks via fp16 ident matmul
    STsb = exp(ST * 0.125) on ACT (fp16, PSUM->SBUF)
    Ops[q, h*65+d] += STsb.T-contract: lhsT=STsb block, rhs=Vplus slice
      (col h*65+64 accumulates the softmax denominator)
  normalize: rec6 = 1/Ops[:, 64::65]; O = Ops * rec6 (broadcast over d)
  OT = PE-transpose(O) fp16; out = OT.T @ w_proj + bias -> DMA out
"""

import numpy as np
from contextlib import ExitStack

import concourse.bass as bass
from concourse import bacc
import concourse.mybir as mybir
import concourse.tile as tile
from concourse.masks import make_identity

F32 = mybir.dt.float32
F16 = mybir.dt.float16

B, T, E, H, D = 256, 256, 384, 6, 64
N_CORES = 8
NB = B // N_CORES  # items per core
EC = E // 128       # 3 e-chunks
MC = (H * D) // 128  # 3 head-dim chunks (2 heads each)
DV = D + 1           # 65: V columns per head incl. ones column
HD = H * DV          # 390

Exp = mybir.ActivationFunctionType.Exp


def build(nb=NB, repeat=1):
    nc = bacc.Bacc("TRN2", debug=False, num_devices=N_CORES)
    x = nc.dram_tensor("x", [nb, T, E], F32, kind="ExternalInput").ap()
    wq = nc.dram_tensor("wq", [H, E, D], F32, kind="ExternalInput").ap()
    wk = nc.dram_tensor("wk", [H, E, D], F32, kind="ExternalInput").ap()
    wv = nc.dram_tensor("wv", [H, E, D], F32, kind="ExternalInput").ap()
    wp = nc.dram_tensor("w_proj", [H * D, E], F32, kind="ExternalInput").ap()
    bp_t = nc.dram_tensor("b_proj", [E], F32, kind="ExternalInput").ap()
    out = nc.dram_tensor("out", [nb, T, E], F32, kind="ExternalOutput").ap()

    with tile.TileContext(nc) as tc, ExitStack() as ctx:
        const = ctx.enter_context(tc.tile_pool(name="const", bufs=1))

        # --- constants ---
        ident = const.tile([128, 128], F32, tag="ident")
        make_identity(nc, ident[:])
        ident16 = const.tile([128, 128], F16, tag="ident16")
        make_identity(nc, ident16[:])
        bias_bc = const.tile([128, E], F32, tag="bias")

        # --- pools ---
        xnp = ctx.enter_context(tc.tile_pool(name="xn", bufs=6))
        xtp = ctx.enter_context(tc.tile_pool(name="xt", bufs=9))
        qkp = ctx.enter_context(tc.tile_pool(name="qk", bufs=18))
        vp = ctx.enter_context(tc.tile_pool(name="v", bufs=12))
        stp = ctx.enter_context(tc.tile_pool(name="st", bufs=12))
        smp = ctx.enter_context(tc.tile_pool(name="sm", bufs=12))
        osp = ctx.enter_context(tc.tile_pool(name="os", bufs=8))
        otp = ctx.enter_context(tc.tile_pool(name="ot", bufs=9))
        obp = ctx.enter_context(tc.tile_pool(name="ob", bufs=6))

        psS = ctx.enter_context(tc.tile_pool(name="psS", bufs=3, space="PSUM"))
        psO = ctx.enter_context(tc.tile_pool(name="psO", bufs=2, space="PSUM"))
        psQ = ctx.enter_context(tc.tile_pool(name="psQ", bufs=3, space="PSUM"))

        assert nb % 2 == 0
        npair = nb // 2

        # prefetch x for pair 0
        xn_tiles = {}

        def load_pair(bp):
            for bi, b in enumerate((2 * bp, 2 * bp + 1)):
                t_ = xnp.tile([128, 2 * E], F32, tag="xn")
                nc.sync.dma_start(
                    t_[:].rearrange("p (c e) -> p c e", c=2),
                    x[b].rearrange("(c p) e -> p c e", c=2))
                xn_tiles[(bp, bi)] = t_

        # x for pair 0 first (PE's first work depends on it), then weights
        load_pair(0)

        # --- weights: load f32, cast to fp16 (casts spread over engines) ---
        # wq/wk/wv: [e(3x128 chunks), h*64+d]; wp: [hd(3x128), e]
        wstage = ctx.enter_context(tc.tile_pool(name="wstage", bufs=4))
        wq_sb, wk_sb, wv_sb, wp_sb = [], [], [], []
        cast_engines = [nc.vector.tensor_copy,
                        lambda o, i: nc.scalar.copy(o, i),
                        nc.gpsimd.tensor_copy]
        ci = 0
        nc.sync.dma_start(bias_bc[:], bp_t.unsqueeze(0).broadcast_to([128, E]))
        for ec in range(EC):
            for (dst, src, tg) in ((wq_sb, wq, "wq"), (wk_sb, wk, "wk"),
                                   (wv_sb, wv, "wv")):
                stg = wstage.tile([128, H * D], F32, tag="stg")
                nc.sync.dma_start(
                    stg[:].rearrange("p (h d) -> p h d", h=H),
                    src.transpose([1, 0, 2])[ec * 128:(ec + 1) * 128, :, :])
                t_ = const.tile([128, H * D], F16, tag=f"{tg}{ec}")
                cast_engines[ci % 3](t_[:], stg[:])
                ci += 1
                dst.append(t_)
            stg = wstage.tile([128, E], F32, tag="stg")
            nc.sync.dma_start(stg[:], wp[ec * 128:(ec + 1) * 128, :])
            t_ = const.tile([128, E], F16, tag=f"wp{ec}")
            cast_engines[ci % 3](t_[:], stg[:])
            ci += 1
            wp_sb.append(t_)

        def emit_tails(bb_, o_pair_):
            """O transposes + projection + bias + output DMA for a pair."""
            for bi in range(2):
                O_sb = o_pair_[bi]
                # O transposes via DMA XBAR (fp16 SBUF->SBUF)
                OT = []
                for c in range(MC):
                    t_ = otp.tile([128, 256], F16, tag="ot")
                    for qc in range(2):
                        nc.sync.dma_start_transpose(
                            t_[:, qc * 128:(qc + 1) * 128],
                            O_sb[qc][:, c * 128:(c + 1) * 128])
                    OT.append(t_)

                ob = obp.tile([128, 2 * E], F32, tag="ob")
                for tcc in range(2):
                    ps = psQ.tile([128, E], F32, tag="psQ")
                    for c in range(MC):
                        nc.tensor.matmul(
                            ps[:],
                            OT[c][:, tcc * 128:(tcc + 1) * 128],
                            wp_sb[c][:],
                            start=(c == 0), stop=(c == MC - 1))
                    nc.vector.tensor_add(
                        ob[:, tcc * E:(tcc + 1) * E], ps[:], bias_bc[:])
                nc.sync.dma_start(
                    out[bb_[bi]].rearrange("(c p) e -> p c e", c=2),
                    ob[:].rearrange("p (c e) -> p c e", c=2))

        rep_ctx = tc.For_i(0, repeat, 1) if repeat > 1 else None
        if rep_ctx is not None:
            rep_ctx.__enter__()

        def prep_xt(bp):
            """Cast x pair to fp16 (Pool), PE-transpose to xT, copy to SBUF."""
            xn16s = []
            for bi in range(2):
                xn = xn_tiles.pop((bp, bi))
                x16 = xnp.tile([128, 2 * E], F16, tag="xn16")
                nc.gpsimd.tensor_copy(x16[:], xn[:])
                xn16s.append(x16)
            xt2 = []
            for ec in range(EC):
                ps = psQ.tile([128, 512], F16, tag="psQ")
                for bi in range(2):
                    xn16 = xn16s[bi]
                    for tcc in range(2):
                        nc.tensor.transpose(
                            ps[:, bi * 256 + tcc * 128:
                               bi * 256 + (tcc + 1) * 128],
                            xn16[:, tcc * E + ec * 128:
                                 tcc * E + ec * 128 + 128],
                            ident16[:])
                t_ = xtp.tile([128, 512], F16, tag="xt")
                nc.vector.tensor_copy(t_[:], ps[:])
                xt2.append(t_)
            return xt2

        xt_cur = prep_xt(0)
        prev_tail = None
        for bp in range(npair):
            bb = (2 * bp, 2 * bp + 1)
            if bp + 1 < npair:
                load_pair(bp + 1)
            xt2 = xt_cur

            # ---- QT / KT for the pair (N=512) fp16, mc-major issue order ----
            QT, KT = [], []
            for mc in range(MC):
                for wi, (w_sb, dst) in enumerate(((wq_sb, QT), (wk_sb, KT))):
                    ps = psQ.tile([128, 512], F32, tag="psQ")
                    for ec in range(EC):
                        nc.tensor.matmul(
                            ps[:],
                            w_sb[ec][:, mc * 128:(mc + 1) * 128],
                            xt2[ec][:],
                            start=(ec == 0), stop=(ec == EC - 1))
                    t_ = qkp.tile([128, 512], F16, tag="qk")
                    if (mc + wi) % 2 == 0:
                        nc.vector.tensor_copy(t_[:], ps[:])
                    else:
                        nc.scalar.copy(t_[:], ps[:])
                    dst.append(t_)

            # ---- V per (item, tchunk): [128, 390] fp16 with ones cols ----
            Vb = []
            for bi in range(2):
                V = []
                for tcc in range(2):
                    ps = psQ.tile([128, H * D], F32, tag="psQ")
                    for ec in range(EC):
                        nc.tensor.matmul(
                            ps[:],
                            xt2[ec][:, bi * 256 + tcc * 128:
                                    bi * 256 + (tcc + 1) * 128],
                            wv_sb[ec][:],
                            start=(ec == 0), stop=(ec == EC - 1))
                    t_ = vp.tile([128, HD], F16, tag="v")
                    tv = t_[:].rearrange("p (h d) -> p h d", h=H)
                    nc.vector.tensor_copy(
                        tv[:, :, 0:D],
                        ps[:].rearrange("p (h d) -> p h d", h=H))
                    nc.gpsimd.memset(tv[:, :, D:DV], 1.0)
                    V.append(t_)
                Vb.append(V)

            # ---- tails of the previous pair (PE covered by QKV above) ----
            if prev_tail is not None:
                emit_tails(*prev_tail)

            # ---- prefetch next pair's xT (PE slot before the head loop) ----
            if bp + 1 < npair:
                xt_cur = prep_xt(bp + 1)

            # ---- heads + normalize + project, per item ----
            o_pair = []
            for bi in range(2):
                qoff = bi * 256
                V = Vb[bi]
                ops = [psO.tile([128, HD], F32, tag="psO", name=f"ops{qc}")
                       for qc in range(2)]

                def emit_ot(h, stsb):
                    sl = slice(h * DV, (h + 1) * DV)
                    nc.tensor.matmul(
                        ops[0][:, sl], stsb[:, 0:128], V[0][:, sl],
                        start=True, stop=True, skip_group_check=True)
                    nc.tensor.matmul(
                        ops[1][:, sl], stsb[:, 128:256], V[0][:, sl],
                        start=True, stop=False, skip_group_check=True)
                    nc.tensor.matmul(
                        ops[1][:, sl], stsb[:, 256:384], V[1][:, sl],
                        start=False, stop=True, skip_group_check=True)

                pending = []  # (h, stsb) whose OT matmuls are not yet issued
                for h in range(H):
                    hc, r0 = h // 2, (h % 2) * 64
                    kt = KT[hc]
                    qt = QT[hc]
                    stps = psS.tile([128, 384], F32, tag="psS")
                    # ST[k0, q0:256]
                    nc.tensor.matmul(
                        stps[:, 0:256],
                        kt[r0:r0 + 64, qoff:qoff + 128],
                        qt[r0:r0 + 64, qoff:qoff + 256],
                        start=True, stop=True, skip_group_check=True)
                    # ST[k1, q1]
                    nc.tensor.matmul(
                        stps[:, 256:384],
                        kt[r0:r0 + 64, qoff + 128:qoff + 256],
                        qt[r0:r0 + 64, qoff + 128:qoff + 256],
                        start=True, stop=True, skip_group_check=True)

                    stsb = stp.tile([128, 384], F16, tag="st")
                    nc.scalar.activation(stsb[:], stps[:], Exp, scale=0.125)
                    # causal mask: zero q < k in both diagonal blocks at once
                    # (strided AP over cols 0:128 and 256:384), on Pool
                    nc.gpsimd.affine_select(
                        out=stsb[:].rearrange("p (b f) -> p b f", b=3)[:, 0::2],
                        in_=stsb[:].rearrange("p (b f) -> p b f", b=3)[:, 0::2],
                        compare_op=mybir.AluOpType.is_ge,
                        fill=0.0,
                        base=0,
                        pattern=[[0, 2], [1, 128]],
                        channel_multiplier=-1,
                    )
                    pending.append((h, stsb))
                    # depth-2 software pipeline: OT of head h-2 issues after
                    # the ST matmuls of head h, so PE has ~2 heads of score
                    # work to cover the exp (ACT) + mask (Pool) latency
                    if len(pending) > 2:
                        emit_ot(*pending.pop(0))
                for p_ in pending:
                    emit_ot(*p_)

                # normalize both q-chunks -> O [q, h*64+d] fp16 packed
                O_sb = []
                for qc in range(2):
                    view = ops[qc][:].rearrange("p (h d) -> p h d", h=H)
                    rec6 = smp.tile([128, H], F32, tag="rec")
                    nc.vector.reciprocal(rec6[:], view[:, :, D])
                    o_t = osp.tile([128, H * D], F16, tag="os")
                    nc.vector.tensor_mul(
                        o_t[:].rearrange("p (h d) -> p h d", h=H),
                        view[:, :, 0:D],
                        rec6[:].unsqueeze(2).broadcast_to([128, H, D]))
                    O_sb.append(o_t)
                o_pair.append(O_sb)
            prev_tail = (bb, o_pair)

        emit_tails(*prev_tail)

        if rep_ctx is not None:
            rep_ctx.__exit__(None, None, None)
    nc.compile()
    return nc


_NC_CACHE = {}


def kernel(x, wq, wk, wv, w_proj, b_proj):
    x = np.ascontiguousarray(np.asarray(x, dtype=np.float32))
    wq = np.ascontiguousarray(np.asarray(wq, dtype=np.float32))
    wk = np.ascontiguousarray(np.asarray(wk, dtype=np.float32))
    wv = np.ascontiguousarray(np.asarray(wv, dtype=np.float32))
    w_proj = np.ascontiguousarray(np.asarray(w_proj, dtype=np.float32))
    b_proj = np.ascontiguousarray(np.asarray(b_proj, dtype=np.float32))

    from concourse.bass_utils import run_bass_kernel_spmd

    if NB not in _NC_CACHE:
        _NC_CACHE[NB] = build(NB)
    nc = _NC_CACHE[NB]

    in_maps = []
    for c in range(N_CORES):
        in_maps.append({
            "x": np.ascontiguousarray(x[c * NB:(c + 1) * NB]),
            "wq": wq, "wk": wk, "wv": wv,
            "w_proj": w_proj, "b_proj": b_proj,
        })
    res = run_bass_kernel_spmd(nc, in_maps, core_ids=list(range(N_CORES)))
    return np.concatenate([r_["out"] for r_ in res.results], axis=0)



# revision 2
# speedup vs baseline: 18.7372x; 18.7372x over previous
"""Multi-head causal attention (B=256,T=256,E=384,H=6,D=64) on 8 trn2 cores.

Sharding: data-parallel over batch (32 items per core), no collectives.

The per-pair work runs inside a hardware For_i loop (items processed in
pairs). This keeps the program ~16x smaller than a fully unrolled kernel:
per-launch instruction-load time dominated the unrolled version (~225 ns
per instruction on first fetch; measured 1.29 ms launch overhead vs 77 us
steady-state body).

Per-pair pipeline (same math as the unrolled baseline):
  x [256,384] --PE transpose--> xT [384, 512] fp16 (pair)
  QT/KT = w.T @ xT   [384, 512] fp16  (heads stacked on partitions)
  V  = xT.T @ wv     [256, 390] fp16  per item, layout [k, h*65+d] with a
       ones column at h*65+64 (fuses row-sum into the AV matmul)
  per head h (per item):
    ST[k,q] = KT_h.T @ QT_h directly (causal: 2 blocks, skip upper rect)
    STsb = exp(ST * 0.125) on ACT (fp16, PSUM->SBUF), causal zeroing on Pool
    Ops[q, h*65+d] += STsb.T-contract: lhsT=STsb block, rhs=Vplus slice
      (col h*65+64 accumulates the softmax denominator)
  normalize: rec6 = 1/Ops[:, 64::65]; O = Ops * rec6 (broadcast over d)
  OT = PE-transpose... no: DMA XBAR transpose fp16; out = OT.T @ w_proj + b
"""

import numpy as np
from contextlib import ExitStack

import concourse.bass as bass
from concourse import bacc
from concourse.bass import ds
import concourse.mybir as mybir
import concourse.tile as tile
from concourse.masks import make_identity

F32 = mybir.dt.float32
F16 = mybir.dt.float16

B, T, E, H, D = 256, 256, 384, 6, 64
N_CORES = 8
NB = B // N_CORES  # items per core
EC = E // 128       # 3 e-chunks
MC = (H * D) // 128  # 3 head-dim chunks (2 heads each)
DV = D + 1           # 65: V columns per head incl. ones column
HD = H * DV          # 390

Exp = mybir.ActivationFunctionType.Exp


def build(nb=NB, repeat=1):
    nc = bacc.Bacc("TRN2", debug=False, num_devices=N_CORES)
    x = nc.dram_tensor("x", [nb, T, E], F32, kind="ExternalInput").ap()
    wq = nc.dram_tensor("wq", [H, E, D], F32, kind="ExternalInput").ap()
    wk = nc.dram_tensor("wk", [H, E, D], F32, kind="ExternalInput").ap()
    wv = nc.dram_tensor("wv", [H, E, D], F32, kind="ExternalInput").ap()
    wp = nc.dram_tensor("w_proj", [H * D, E], F32, kind="ExternalInput").ap()
    bp_t = nc.dram_tensor("b_proj", [E], F32, kind="ExternalInput").ap()
    out = nc.dram_tensor("out", [nb, T, E], F32, kind="ExternalOutput").ap()

    # [b, p, c, e] views with 128-partition tiles (c = 2 t-chunks)
    x_v = x.rearrange("b (c p) e -> b p c e", c=2)
    out_v = out.rearrange("b (c p) e -> b p c e", c=2)

    with tile.TileContext(nc) as tc, ExitStack() as ctx:
        const = ctx.enter_context(tc.tile_pool(name="const", bufs=1))

        # --- constants ---
        ident16 = const.tile([128, 128], F16, tag="ident16")
        make_identity(nc, ident16[:])
        bias_bc = const.tile([128, E], F32, tag="bias")

        # --- pools ---
        xnp = ctx.enter_context(tc.tile_pool(name="xn", bufs=2))
        xtp = ctx.enter_context(tc.tile_pool(name="xt", bufs=3))
        qkp = ctx.enter_context(tc.tile_pool(name="qk", bufs=6))
        vp = ctx.enter_context(tc.tile_pool(name="v", bufs=4))
        stp = ctx.enter_context(tc.tile_pool(name="st", bufs=4))
        smp = ctx.enter_context(tc.tile_pool(name="sm", bufs=4))
        osp = ctx.enter_context(tc.tile_pool(name="os", bufs=4))
        otp = ctx.enter_context(tc.tile_pool(name="ot", bufs=3))
        obp = ctx.enter_context(tc.tile_pool(name="ob", bufs=2))

        psS = ctx.enter_context(tc.tile_pool(name="psS", bufs=2, space="PSUM"))
        psO = ctx.enter_context(tc.tile_pool(name="psO", bufs=2, space="PSUM"))
        psQ = ctx.enter_context(tc.tile_pool(name="psQ", bufs=3, space="PSUM"))

        assert nb % 2 == 0

        # --- weights: load f32, cast to fp16 (casts spread over engines) ---
        wstage = ctx.enter_context(tc.tile_pool(name="wstage", bufs=4))
        wq_sb, wk_sb, wv_sb, wp_sb = [], [], [], []
        cast_engines = [nc.vector.tensor_copy,
                        lambda o, i: nc.scalar.copy(o, i),
                        nc.gpsimd.tensor_copy]
        ci = 0
        nc.sync.dma_start(bias_bc[:], bp_t.unsqueeze(0).broadcast_to([128, E]))
        for ec in range(EC):
            for (dst, src, tg) in ((wq_sb, wq, "wq"), (wk_sb, wk, "wk"),
                                   (wv_sb, wv, "wv")):
                stg = wstage.tile([128, H * D], F32, tag="stg")
                nc.sync.dma_start(
                    stg[:].rearrange("p (h d) -> p h d", h=H),
                    src.transpose([1, 0, 2])[ec * 128:(ec + 1) * 128, :, :])
                t_ = const.tile([128, H * D], F16, tag=f"{tg}{ec}")
                cast_engines[ci % 3](t_[:], stg[:])
                ci += 1
                dst.append(t_)
            stg = wstage.tile([128, E], F32, tag="stg")
            nc.sync.dma_start(stg[:], wp[ec * 128:(ec + 1) * 128, :])
            t_ = const.tile([128, E], F16, tag=f"wp{ec}")
            cast_engines[ci % 3](t_[:], stg[:])
            ci += 1
            wp_sb.append(t_)

        def emit_pair(b0):
            """Full pipeline for items (b0, b0+1); b0 may be a loop var."""
            # ---- load x pair ----
            xns = []
            for bi in range(2):
                t_ = xnp.tile([128, 2 * E], F32, tag=f"xn{bi}")
                nc.sync.dma_start(
                    t_[:].rearrange("p (c e) -> p c e", c=2),
                    x_v[ds(b0 + bi, 1)].rearrange("o p c e -> (o p) c e"))
                xns.append(t_)

            # ---- cast to fp16 (Pool), PE-transpose to xT ----
            xn16s = []
            for bi in range(2):
                x16 = xnp.tile([128, 2 * E], F16, tag=f"xn16{bi}")
                nc.gpsimd.tensor_copy(x16[:], xns[bi][:])
                xn16s.append(x16)
            xt2 = []
            for ec in range(EC):
                ps = psQ.tile([128, 512], F16, tag="psQ")
                for bi in range(2):
                    xn16 = xn16s[bi]
                    for tcc in range(2):
                        nc.tensor.transpose(
                            ps[:, bi * 256 + tcc * 128:
                               bi * 256 + (tcc + 1) * 128],
                            xn16[:, tcc * E + ec * 128:
                                 tcc * E + ec * 128 + 128],
                            ident16[:])
                t_ = xtp.tile([128, 512], F16, tag="xt")
                nc.vector.tensor_copy(t_[:], ps[:])
                xt2.append(t_)

            # ---- QT / KT for the pair (N=512) fp16, mc-major issue order ----
            QT, KT = [], []
            for mc in range(MC):
                for wi, (w_sb, dst) in enumerate(((wq_sb, QT), (wk_sb, KT))):
                    ps = psQ.tile([128, 512], F32, tag="psQ")
                    for ec in range(EC):
                        nc.tensor.matmul(
                            ps[:],
                            w_sb[ec][:, mc * 128:(mc + 1) * 128],
                            xt2[ec][:],
                            start=(ec == 0), stop=(ec == EC - 1))
                    t_ = qkp.tile([128, 512], F16, tag="qk")
                    if (mc + wi) % 2 == 0:
                        nc.vector.tensor_copy(t_[:], ps[:])
                    else:
                        nc.scalar.copy(t_[:], ps[:])
                    dst.append(t_)

            # ---- V per (item, tchunk): [128, 390] fp16 with ones cols ----
            Vb = []
            for bi in range(2):
                V = []
                for tcc in range(2):
                    ps = psQ.tile([128, H * D], F32, tag="psQ")
                    for ec in range(EC):
                        nc.tensor.matmul(
                            ps[:],
                            xt2[ec][:, bi * 256 + tcc * 128:
                                    bi * 256 + (tcc + 1) * 128],
                            wv_sb[ec][:],
                            start=(ec == 0), stop=(ec == EC - 1))
                    t_ = vp.tile([128, HD], F16, tag="v")
                    tv = t_[:].rearrange("p (h d) -> p h d", h=H)
                    nc.vector.tensor_copy(
                        tv[:, :, 0:D],
                        ps[:].rearrange("p (h d) -> p h d", h=H))
                    nc.gpsimd.memset(tv[:, :, D:DV], 1.0)
                    V.append(t_)
                Vb.append(V)

            # ---- heads + normalize, per item ----
            o_pair = []
            for bi in range(2):
                qoff = bi * 256
                V = Vb[bi]
                ops = [psO.tile([128, HD], F32, tag="psO", name=f"ops{qc}")
                       for qc in range(2)]

                def emit_ot(h, stsb):
                    sl = slice(h * DV, (h + 1) * DV)
                    nc.tensor.matmul(
                        ops[0][:, sl], stsb[:, 0:128], V[0][:, sl],
                        start=True, stop=True, skip_group_check=True)
                    nc.tensor.matmul(
                        ops[1][:, sl], stsb[:, 128:256], V[0][:, sl],
                        start=True, stop=False, skip_group_check=True)
                    nc.tensor.matmul(
                        ops[1][:, sl], stsb[:, 256:384], V[1][:, sl],
                        start=False, stop=True, skip_group_check=True)

                pending = []
                for h in range(H):
                    hc, r0 = h // 2, (h % 2) * 64
                    kt = KT[hc]
                    qt = QT[hc]
                    stps = psS.tile([128, 384], F32, tag="psS")
                    nc.tensor.matmul(
                        stps[:, 0:256],
                        kt[r0:r0 + 64, qoff:qoff + 128],
                        qt[r0:r0 + 64, qoff:qoff + 256],
                        start=True, stop=True, skip_group_check=True)
                    nc.tensor.matmul(
                        stps[:, 256:384],
                        kt[r0:r0 + 64, qoff + 128:qoff + 256],
                        qt[r0:r0 + 64, qoff + 128:qoff + 256],
                        start=True, stop=True, skip_group_check=True)

                    stsb = stp.tile([128, 384], F16, tag="st")
                    nc.scalar.activation(stsb[:], stps[:], Exp, scale=0.125)
                    # causal mask: zero q < k in both diagonal blocks at once
                    nc.gpsimd.affine_select(
                        out=stsb[:].rearrange("p (b f) -> p b f", b=3)[:, 0::2],
                        in_=stsb[:].rearrange("p (b f) -> p b f", b=3)[:, 0::2],
                        compare_op=mybir.AluOpType.is_ge,
                        fill=0.0,
                        base=0,
                        pattern=[[0, 2], [1, 128]],
                        channel_multiplier=-1,
                    )
                    pending.append((h, stsb))
                    # depth-2 software pipeline: OT of head h-2 issues after
                    # the ST matmuls of head h
                    if len(pending) > 2:
                        emit_ot(*pending.pop(0))
                for p_ in pending:
                    emit_ot(*p_)

                # normalize both q-chunks -> O [q, h*64+d] fp16 packed
                O_sb = []
                for qc in range(2):
                    view = ops[qc][:].rearrange("p (h d) -> p h d", h=H)
                    rec6 = smp.tile([128, H], F32, tag="rec")
                    nc.vector.reciprocal(rec6[:], view[:, :, D])
                    o_t = osp.tile([128, H * D], F16, tag="os")
                    nc.vector.tensor_mul(
                        o_t[:].rearrange("p (h d) -> p h d", h=H),
                        view[:, :, 0:D],
                        rec6[:].unsqueeze(2).broadcast_to([128, H, D]))
                    O_sb.append(o_t)
                o_pair.append(O_sb)

            # ---- O transposes + projection + bias + output DMA ----
            for bi in range(2):
                O_sb = o_pair[bi]
                OT = []
                for c in range(MC):
                    t_ = otp.tile([128, 256], F16, tag="ot")
                    for qc in range(2):
                        nc.sync.dma_start_transpose(
                            t_[:, qc * 128:(qc + 1) * 128],
                            O_sb[qc][:, c * 128:(c + 1) * 128])
                    OT.append(t_)

                ob = obp.tile([128, 2 * E], F32, tag="ob")
                for tcc in range(2):
                    ps = psQ.tile([128, E], F32, tag="psQ")
                    for c in range(MC):
                        nc.tensor.matmul(
                            ps[:],
                            OT[c][:, tcc * 128:(tcc + 1) * 128],
                            wp_sb[c][:],
                            start=(c == 0), stop=(c == MC - 1))
                    nc.vector.tensor_add(
                        ob[:, tcc * E:(tcc + 1) * E], ps[:], bias_bc[:])
                nc.sync.dma_start(
                    out_v[ds(b0 + bi, 1)].rearrange("o p c e -> (o p) c e"),
                    ob[:].rearrange("p (c e) -> p c e", c=2))

        with tc.For_i(0, nb, 2) as b0:
            emit_pair(b0)

    nc.compile()
    return nc


_NC_CACHE = {}


def kernel(x, wq, wk, wv, w_proj, b_proj):
    x = np.ascontiguousarray(np.asarray(x, dtype=np.float32))
    wq = np.ascontiguousarray(np.asarray(wq, dtype=np.float32))
    wk = np.ascontiguousarray(np.asarray(wk, dtype=np.float32))
    wv = np.ascontiguousarray(np.asarray(wv, dtype=np.float32))
    w_proj = np.ascontiguousarray(np.asarray(w_proj, dtype=np.float32))
    b_proj = np.ascontiguousarray(np.asarray(b_proj, dtype=np.float32))

    from concourse.bass_utils import run_bass_kernel_spmd

    if NB not in _NC_CACHE:
        _NC_CACHE[NB] = build(NB)
    nc = _NC_CACHE[NB]

    in_maps = []
    for c in range(N_CORES):
        in_maps.append({
            "x": np.ascontiguousarray(x[c * NB:(c + 1) * NB]),
            "wq": wq, "wk": wk, "wv": wv,
            "w_proj": w_proj, "b_proj": b_proj,
        })
    res = run_bass_kernel_spmd(nc, in_maps, core_ids=list(range(N_CORES)))
    return np.concatenate([r_["out"] for r_ in res.results], axis=0)
